# revision 4
# baseline (speedup 1.0000x reference)
"""Trainium2 Bass kernel for nn_Dependence_Learning (sparse_attention) — v2.

L-sharded design: each of 8 cores owns 8 image rows (512 pixels) of every
(batch, channel) pair.  Channels live on partitions (2 banks of 128).

  conv1/conv2: lhsT = w.T [in_ch part, out_ch], rhs = x [in_ch part, cols]
  BN stats: per-channel = per-partition bn_stats/bn_aggr over the owned
  region, then ONE AllGather of (mean, var) x 4 slots and a local combine.
  tri maps stored fp16 in a 65-col padded row layout (shared pad column
  gives correct zero for +-1 column shifts).

  Attention: the reference's scrambled reshape makes softmax run over
  groups of 9 consecutive positions of the flat space p = 4096 t + l.
  Per core the group phase r = (t - k) mod 9.  The loop runs over the
  PHASE rho (static geometry on every core); the shift used is
  t = (rho + k) mod 9, supplied per-core as a register offset table
  ("offt") that biases the tri / x source APs.  Cross-shift seam values
  (first/last <=8 positions, cores 0/7) come from a host-prepared patch
  strip ("xp") run through the same conv+BN path ("hp"), patched into the
  F tiles with one strided copy.

  Engine split per (rho, ob): Act: F overcopies + seam patches + exp;
  Pool(gpsimd): the two center-broadcast muls (t2's last batch on DVE);
  DVE: add, group-sum, recip, normalize, x-mul, BN applies; PE: the
  final 1x1 conv FUSED into the loop (yf = wf . sum_t Pt is linear, so
  each Pt feeds wf-matmuls accumulating straight into the final PSUM).

  Then yf evict + bn_stats, second AllGather for BNf stats, relu, out.
"""

import numpy as np
from contextlib import ExitStack

import concourse.bass as bass
import concourse.bacc as bacc
import concourse.tile as tile
import concourse.mybir as mybir
from concourse.bass_utils import run_bass_kernel_spmd

F32 = mybir.dt.float32
F32R = mybir.dt.float32r
F16 = mybir.dt.float16
I32 = mybir.dt.int32
AF = mybir.ActivationFunctionType
AX = mybir.AxisListType

NCORES = 8
B = 4
C = 256
HH = 64
L = HH * HH            # 4096
P = 128
NB = 2                 # channel banks
RO = 8                 # owned rows per core
LC = RO * HH           # 512 owned pixels
RH = 12                # slab rows (halo 2 each side)
RX = 10                # x_att rows (halo 1 each side)
CC = B * RH * HH       # 3072 conv cols per bank
TRW = 1 + RH * 65      # 781   tri row-padded width per b
XAW = 1 + RX * 65      # 651
EPS = 1e-5
EXPB = -30.0
NPC = B * 9 * 16       # 576 patch-strip cols


def _shift(t):
    return t // 3 - 1, t % 3 - 1


def _geom(rho):
    s = (1 - rho) % 9
    ng = (rho + 512 + s) // 9
    return s, ng


def build_program():
    nc = bacc.Bacc("TRN2", target_bir_lowering=False, num_devices=NCORES)

    xs_t = nc.dram_tensor("xs", [NB * P, CC], F32R, kind="ExternalInput")
    xp_t = nc.dram_tensor("xp", [NB * P, NPC], F32R, kind="ExternalInput")
    wts_t = nc.dram_tensor("wts", [NB * P, 3 * C], F32R, kind="ExternalInput")
    bnq_t = nc.dram_tensor("bnq", [NB * P, 6], F32, kind="ExternalInput")
    hm_t = nc.dram_tensor("hmask", [P, 2], F32, kind="ExternalInput")
    hpm_t = nc.dram_tensor("hpmv", [P, 144], F16, kind="ExternalInput")
    offt_t = nc.dram_tensor("offt", [1, 18], I32, kind="ExternalInput")
    out_t = nc.dram_tensor("out", [NB * P, B * LC], F32, kind="ExternalOutput")

    with tile.TileContext(nc) as tc, ExitStack() as top:
        consts = top.enter_context(tc.tile_pool(name="consts", bufs=1))
        persist = top.enter_context(tc.tile_pool(name="persist", bufs=1))
        tiny = top.enter_context(tc.tile_pool(name="tiny", bufs=4))
        dram = top.enter_context(tc.tile_pool(name="dram", bufs=1, space="DRAM"))

        # ---- weights / constants ----
        # all conv weights in two [128, 768] tiles (one DMA each):
        # cols = (w1|w2|wf) * 256 + ob * 128 + co
        wbig = []
        for ib in range(NB):
            wb = consts.tile([P, 3 * C], F32R, tag=f"wbig{ib}",
                             name=f"wbig{ib}")
            nc.scalar.dma_start(out=wb[:, :],
                                in_=wts_t[ib * P:(ib + 1) * P, :])
            wbig.append(wb)
        _wi = {"w1": 0, "w2": 1, "wf": 2}

        def wt(key):
            name, ib, ob = key
            j = _wi[name] * C + ob * P
            return wbig[ib][:, j:j + P]
        # fp16 wf blocks for the fused final-conv accumulation
        wf16 = {}
        for ib in range(NB):
            for oo in range(NB):
                w16 = consts.tile([P, P], F16, tag=f"wf16_{ib}{oo}",
                                  name=f"wf16_{ib}{oo}")
                nc.vector.tensor_copy(out=w16[:, :],
                                      in_=wt(("wf", ib, oo)).bitcast(F32))
                wf16[(ib, oo)] = w16
        bnc = []
        for ob in range(NB):
            c_ = consts.tile([P, 6], F32, tag=f"bnc{ob}", name=f"bnc{ob}")
            nc.sync.dma_start(out=c_[:, :], in_=bnq_t[ob * P:(ob + 1) * P, :])
            bnc.append(c_)
        hmask = consts.tile([P, 2], F32, tag="hmask")
        nc.sync.dma_start(out=hmask[:, :], in_=hm_t[:, :])
        hpm = consts.tile([P, 144], F16, tag="hpm")
        nc.gpsimd.dma_start(out=hpm[:, :], in_=hpm_t[:, :])
        offs = consts.tile([1, 18], I32, tag="offs")
        nc.sync.dma_start(out=offs[:, :], in_=offt_t[:, :])
        eps_c = consts.tile([P, 1], F32, tag="eps_c")
        nc.vector.memset(eps_c[:, :], EPS)
        expb_c = consts.tile([P, 1], F32, tag="expb_c")
        nc.vector.memset(expb_c[:, :], EXPB)

        # ---- persistent attention-phase buffers ----
        tri = [persist.tile([P, NB, B, TRW], F16, tag=f"tri{m}",
                            name=f"tri{m}") for m in range(2)]
        x_att = persist.tile([P, NB, B, XAW], F16, tag="x_att")
        hp = [persist.tile([P, NB, B, 9, 16], F16, tag=f"hp{m}",
                           name=f"hp{m}") for m in range(2)]
        acol = persist.tile([P, 4], F32, tag="acol")
        bcol = persist.tile([P, 4], F32, tag="bcol")

        statd = dram.tile([P, 8], F32, tag="statd")
        statg = dram.tile([NCORES * P, 8], F32, tag="statg")
        statd2 = dram.tile([P, 4], F32, tag="statd2")
        statg2 = dram.tile([NCORES * P, 4], F32, tag="statg2")

        def combine_stats(sg, nslots, gmean, gvar):
            """sg [P, nslots(mean,var interleaved j), 8 cores] -> global."""
            msq = tiny.tile([P, nslots, 8], F32, tag="msq")
            mv = sg[:, :, :]  # [P, 2*nslots, 8]
            mean_v = bass.AP(tensor=sg.tensor, offset=sg.offset,
                             ap=[[2 * nslots * 8, P], [16, nslots], [1, 8]])
            var_v = bass.AP(tensor=sg.tensor, offset=sg.offset + 8,
                            ap=[[2 * nslots * 8, P], [16, nslots], [1, 8]])
            nc.vector.tensor_mul(msq[:, :, :], mean_v, mean_v)
            nc.vector.tensor_add(msq[:, :, :], msq[:, :, :], var_v)
            nc.vector.reduce_sum(out=gmean[:, :], in_=mean_v, axis=AX.X)
            nc.vector.reduce_sum(out=gvar[:, :], in_=msq[:, :, :], axis=AX.X)
            nc.vector.tensor_scalar_mul(gmean[:, :], gmean[:, :], 1.0 / 8)
            nc.vector.tensor_scalar_mul(gvar[:, :], gvar[:, :], 1.0 / 8)
            gm2 = tiny.tile([P, nslots], F32, tag="gm2")
            nc.vector.tensor_mul(gm2[:, :], gmean[:, :], gmean[:, :])
            nc.vector.tensor_sub(gvar[:, :], gvar[:, :], gm2[:, :])

        def bn_coeffs(gmean, gvar, acol_, bcol_, gcols, becols):
            """acol = g/sqrt(var+eps); bcol = be - mean*acol (all [P, n])."""
            n = gvar.shape[1]
            sd = tiny.tile([P, n], F32, tag="sd")
            nc.scalar.activation(out=sd[:, :], in_=gvar[:, :], func=AF.Sqrt,
                                 bias=eps_c[:, :])
            nc.vector.reciprocal(sd[:, :], sd[:, :])
            nc.vector.tensor_mul(acol_[:, :], gcols, sd[:, :])
            tmp = tiny.tile([P, n], F32, tag="tmpc")
            nc.vector.tensor_mul(tmp[:, :], gmean[:, :], acol_[:, :])
            nc.vector.tensor_sub(bcol_[:, :], becols, tmp[:, :])

        # ================= phase 1: convs + BN stats ==================
        with ExitStack() as s1:
            rhsp = s1.enter_context(tc.tile_pool(name="rhs", bufs=4))
            psump = s1.enter_context(
                tc.tile_pool(name="psum", bufs=2, space="PSUM"))
            ybuf = s1.enter_context(tc.tile_pool(name="ybuf", bufs=1))
            y = {}
            for m in range(2):
                for ob in range(NB):
                    y[(m, ob)] = ybuf.tile([P, CC], F16, tag=f"y{m}{ob}",
                                           name=f"y{m}{ob}")
            hpraw = [ybuf.tile([P, NB, B, 9, 16], F16, tag=f"hpr{m}",
                               name=f"hpr{m}") for m in range(2)]

            xc = [rhsp.tile([P, CC], F32R, tag=f"xc{ib}", name=f"xc{ib}")
                  for ib in range(NB)]
            for ib in range(NB):
                for h in range(3):
                    nc.sync.dma_start(
                        out=xc[ib][:, h * 1024:(h + 1) * 1024],
                        in_=xs_t[ib * P:(ib + 1) * P,
                                 h * 1024:(h + 1) * 1024])

            NCH = 6
            CW = CC // NCH  # 512
            for chk in range(NCH):
                sl = slice(chk * CW, (chk + 1) * CW)
                for m, wname in ((0, "w1"), (1, "w2")):
                    for ob in range(NB):
                        ps = psump.tile([P, CW], F32, tag="ps", name="ps")
                        for ib in range(NB):
                            nc.tensor.matmul(
                                ps[:, :], wt((wname, ib, ob)),
                                xc[ib][:, sl], start=(ib == 0), stop=(ib == 1),
                                tile_position=(0, 0))
                        nc.scalar.activation(
                            out=y[(m, ob)][:, sl],
                            in_=ps[:, :], func=AF.Copy)

            # x_att: cast slab rows 1..10 into padded layout (early: only
            # needs xc), plus all pad-position zeroing for x_att and tri
            for ib in range(NB):
                nc.vector.tensor_copy(
                    out=bass.AP(
                        tensor=x_att.tensor,
                        offset=x_att.offset + ib * B * XAW + 1,
                        ap=[[NB * B * XAW, P], [XAW, B], [65, RX], [1, HH]]),
                    in_=bass.AP(tensor=xc[ib].tensor,
                                offset=xc[ib].offset + HH,
                                ap=[[CC, P], [768, B], [64, RX],
                                    [1, HH]]).bitcast(F32))
            nc.vector.memset(
                bass.AP(tensor=x_att.tensor, offset=x_att.offset,
                        ap=[[NB * B * XAW, P], [B * XAW, NB], [XAW, B],
                            [65, RX + 1]]), 0.0)
            for m in range(2):
                nc.vector.memset(
                    bass.AP(tensor=tri[m].tensor, offset=tri[m].offset,
                            ap=[[NB * B * TRW, P], [B * TRW, NB], [TRW, B],
                                [65, RH + 1]]), 0.0)

            # patch strips: same convs on xp
            rp = []
            for ib in range(NB):
                r = rhsp.tile([P, NPC], F32R, tag="rhsp", name=f"rhsp{ib}")
                nc.gpsimd.dma_start(out=r[:, :],
                                    in_=xp_t[ib * P:(ib + 1) * P, :])
                rp.append(r)
            for m, wname in ((0, "w1"), (1, "w2")):
                for ob in range(NB):
                    for half in range(2):
                        sl = slice(half * 288, (half + 1) * 288)
                        ps = psump.tile([P, 288], F32, tag="psp", name="psp")
                        for ib in range(NB):
                            nc.tensor.matmul(
                                ps[:, :], wt((wname, ib, ob)),
                                rp[ib][:, sl], start=(ib == 0), stop=(ib == 1),
                                tile_position=(0, 0))
                        nc.scalar.activation(
                            out=hpraw[m][:, ob, 2 * half:2 * half + 2]
                            .rearrange("p b t j -> p (b t j)"),
                            in_=ps[:, :], func=AF.Copy)

            # ---- local BN stats ----
            stt = persist.tile([P, 4, 4, 6], F32, tag="stt")
            msv = persist.tile([P, 4, 2], F32, tag="msv")
            for m in range(2):
                for ob in range(NB):
                    slot = 2 * m + ob
                    for b in range(B):
                        nc.vector.bn_stats(
                            out=stt[:, slot, b, :],
                            in_=y[(m, ob)][:, b * 768 + 128:b * 768 + 640])
                    nc.vector.bn_aggr(out=msv[:, slot, :],
                                      in_=stt[:, slot, :, :])
            nc.sync.dma_start(out=statd[:, :],
                              in_=msv[:, :, :].rearrange("p s v -> p (s v)"))
            nc.gpsimd.collective_compute(
                "AllGather", mybir.AluOpType.bypass,
                replica_groups=[list(range(NCORES))],
                ins=[statd[:, :].opt()], outs=[statg[:, :].opt()])
            sg = persist.tile([P, 8, 8], F32, tag="sg")
            nc.sync.dma_start(
                out=sg[:, :, :],
                in_=bass.AP(tensor=statg.tensor, offset=statg.offset,
                            ap=[[8, P], [1, 8], [8 * P, 8]]))
            gmean = persist.tile([P, 4], F32, tag="gmean")
            gvar = persist.tile([P, 4], F32, tag="gvar")
            combine_stats(sg, 4, gmean, gvar)
            # g/be columns per slot: slot=2m+ob -> bnc[ob][:, 2m], [:, 2m+1]
            gq = persist.tile([P, 4], F32, tag="gq")
            bq = persist.tile([P, 4], F32, tag="bq")
            for m in range(2):
                for ob in range(NB):
                    slot = 2 * m + ob
                    nc.vector.tensor_copy(out=gq[:, slot:slot + 1],
                                          in_=bnc[ob][:, 2 * m:2 * m + 1])
                    nc.vector.tensor_copy(out=bq[:, slot:slot + 1],
                                          in_=bnc[ob][:, 2 * m + 1:2 * m + 2])
            bn_coeffs(gmean, gvar, acol, bcol, gq[:, :], bq[:, :])

            # ---- BN apply + relu -> tri (padded), hp ----
            # Halo rows at the global image edge must be 0 (reference pads
            # tri with zeros).  relu(hm*(a*y+b)) == hm*relu(a*y+b) for
            # hm in {0,1}, so masked coefficients fold the mask into the
            # apply: 3 row bands (top halo, interior, bottom halo).
            acolH = persist.tile([P, 4], F32, tag="acolH")
            bcolH = persist.tile([P, 4], F32, tag="bcolH")
            acolB = persist.tile([P, 4], F32, tag="acolB")
            bcolB = persist.tile([P, 4], F32, tag="bcolB")
            nc.vector.tensor_scalar_mul(acolH[:, :], acol[:, :], hmask[:, 0:1])
            nc.vector.tensor_scalar_mul(bcolH[:, :], bcol[:, :], hmask[:, 0:1])
            nc.vector.tensor_scalar_mul(acolB[:, :], acol[:, :], hmask[:, 1:2])
            nc.vector.tensor_scalar_mul(bcolB[:, :], bcol[:, :], hmask[:, 1:2])
            for ob in range(NB):
                for m in range(2):
                    slot = 2 * m + ob
                    # BN apply on DVE (tensor_scalar affine at fp16 4x,
                    # then relu) so Act is free for the first overcopies
                    for (r0_, nr_, ac_, bc_) in (
                            (2, RH - 4, acol, bcol),
                            (0, 2, acolH, bcolH),
                            (RH - 2, 2, acolB, bcolB)):
                        dst = bass.AP(
                            tensor=tri[m].tensor,
                            offset=(tri[m].offset + ob * B * TRW + 1
                                    + r0_ * 65),
                            ap=[[NB * B * TRW, P], [TRW, B], [65, nr_],
                                [1, HH]])
                        nc.vector.tensor_scalar(
                            out=dst,
                            in0=bass.AP(
                                tensor=y[(m, ob)].tensor,
                                offset=y[(m, ob)].offset + r0_ * HH,
                                ap=[[CC, P], [768, B], [64, nr_], [1, HH]]),
                            scalar1=ac_[:, slot:slot + 1],
                            scalar2=bc_[:, slot:slot + 1],
                            op0=mybir.AluOpType.mult,
                            op1=mybir.AluOpType.add)
                        nc.vector.tensor_scalar_max(dst, dst, 0.0)
                    nc.scalar.activation(
                        out=hp[m][:, ob], in_=hpraw[m][:, ob], func=AF.Relu,
                        bias=bcol[:, slot:slot + 1],
                        scale=acol[:, slot:slot + 1])
                    # zero the patch slots whose source pixel is outside
                    # the image (reference zero-padding)
                    nc.vector.tensor_mul(
                        hp[m][:, ob],
                        hp[m][:, ob],
                        bass.AP(tensor=hpm.tensor, offset=hpm.offset,
                                ap=[[144, P], [0, B], [16, 9], [1, 16]]))
        # ================= phase 2: attention ==================
        # psF accumulates the FINAL conv directly: yf = wf . sum_t Pt
        # (linear), so each iteration's Pt feeds wf-matmuls straight into
        # the final-conv PSUM — no separate pre accumulation/eviction.
        psumA = top.enter_context(
            tc.tile_pool(name="psumA", bufs=1, space="PSUM"))
        psF = [psumA.tile([P, B * LC], F32, tag=f"psF{oo}",
                          name=f"psF{oo}") for oo in range(NB)]
        with ExitStack() as s2:
            fpool = s2.enter_context(tc.tile_pool(name="fpool", bufs=3))
            tpool = s2.enter_context(tc.tile_pool(name="tpool", bufs=3))
            npool = s2.enter_context(tc.tile_pool(name="npool", bufs=3))

            for rho in range(9):
                s_, ng = _geom(rho)
                w9 = 9 * ng
                # per-core shift offsets (registers)
                shoff_a = nc.scalar.value_load(offs[0:1, rho:rho + 1])
                shoff_v = nc.vector.value_load(offs[0:1, rho:rho + 1])
                hoff_a = nc.scalar.value_load(offs[0:1, 9 + rho:10 + rho])
                for ob in range(NB):
                    Fc = []
                    for m in range(2):
                        F_ = fpool.tile([P, B, 640], F16, tag=f"F{m}",
                                        name=f"F{m}c")
                        # overcopy: 10 shifted rows per b (Act)
                        src = bass.AP(
                            tensor=tri[m].tensor,
                            offset=tri[m].offset + ob * B * TRW + shoff_a,
                            ap=[[NB * B * TRW, P], [TRW, B], [65, 10],
                                [1, 64]],
                            dep_tracking_offset=(tri[m].offset
                                                 + ob * B * TRW))
                        dst = F_[:, :, :].rearrange(
                            "p b (r c) -> p b r c", c=64)
                        nc.scalar.activation(out=dst, in_=src, func=AF.Copy)
                        # seam patch (Act): head [56:64) + tail [576:584)
                        nc.scalar.activation(
                            out=bass.AP(tensor=F_.tensor,
                                        offset=F_.offset + 56,
                                        ap=[[B * 640, P], [640, B], [520, 2],
                                            [1, 8]]),
                            in_=bass.AP(
                                tensor=hp[m].tensor,
                                offset=hp[m].offset + ob * B * 144 + hoff_a,
                                ap=[[NB * B * 144, P], [144, B], [8, 2],
                                    [1, 8]],
                                dep_tracking_offset=(hp[m].offset
                                                     + ob * B * 144)),
                            func=AF.Copy)
                        Fc.append(F_)
                    t1 = tpool.tile([P, B, 528], F16, tag="t1", name="t1")
                    t2 = tpool.tile([P, B, 528], F16, tag="t2", name="t2")
                    lgb = tpool.tile([P, B, 528], F16, tag="lgb", name="lgb")
                    Ne = npool.tile([P, B, 528], F32, tag="Ne", name="Ne")
                    Z = npool.tile([P, B, 64], F32, tag="Z", name="Z")
                    R = npool.tile([P, B, 64], F32, tag="R", name="R")
                    NN = npool.tile([P, B, 528], F16, tag="NN", name="NN")
                    Pt = npool.tile([P, B, LC], F16, tag="Pt", name="Pt")

                    def gv(tl, off, b0, nb):
                        return bass.AP(
                            tensor=tl.tensor,
                            offset=tl.offset + tl.shape[2] * b0 + off,
                            ap=[[tl.shape[1] * tl.shape[2], P],
                                [tl.shape[2], nb], [9, ng], [1, 9]])

                    def cbv(tl, off, b0, nb):
                        return bass.AP(
                            tensor=tl.tensor,
                            offset=tl.offset + tl.shape[2] * b0 + off,
                            ap=[[tl.shape[1] * tl.shape[2], P],
                                [tl.shape[2], nb], [9, ng], [0, 9]])

                    # last phase runs per-b to shorten the drain chain
                    bsl = [(b, 1) for b in range(B)] if rho == 8 else [(0, B)]
                    for (b0, nb) in bsl:
                        # center-broadcast muls: t1 on Pool; t2 split
                        # Pool (b<3) / DVE (b=3) to balance the pacer
                        nc.gpsimd.tensor_mul(
                            gv(t1, 0, b0, nb), gv(Fc[0], 64 - rho, b0, nb),
                            cbv(Fc[1], 64 - rho + 4, b0, nb))
                        for (c0, nb2, eng) in (((b0, min(nb, 3), nc.gpsimd),
                                                (3, 1, nc.vector))
                                               if (b0 == 0 and nb == B) else
                                               ((b0, nb,
                                                 nc.vector if b0 == 3
                                                 else nc.gpsimd),)):
                            eng.tensor_mul(
                                gv(t2, 0, c0, nb2),
                                gv(Fc[1], 64 - rho, c0, nb2),
                                cbv(Fc[0], 64 - rho + 4, c0, nb2))
                        nc.vector.tensor_add(
                            lgb[:, b0:b0 + nb, :w9],
                            t1[:, b0:b0 + nb, :w9], t2[:, b0:b0 + nb, :w9])
                        nc.scalar.activation(out=Ne[:, b0:b0 + nb, :w9],
                                             in_=lgb[:, b0:b0 + nb, :w9],
                                             func=AF.Exp, bias=expb_c[:, :])
                        nc.vector.reduce_sum(
                            out=Z[:, b0:b0 + nb, :ng],
                            in_=Ne[:, b0:b0 + nb, :w9].rearrange(
                                "p b (g s) -> p b g s", s=9),
                            axis=AX.X)
                        nc.vector.reciprocal(R[:, b0:b0 + nb, :ng],
                                             Z[:, b0:b0 + nb, :ng])
                        nc.vector.tensor_mul(
                            NN[:, b0:b0 + nb, :w9].rearrange(
                                "p b (g s) -> p b g s", s=9),
                            Ne[:, b0:b0 + nb, :w9].rearrange(
                                "p b (g s) -> p b g s", s=9),
                            bass.AP(tensor=R.tensor,
                                    offset=R.offset + 64 * b0,
                                    ap=[[B * 64, P], [64, nb], [1, ng],
                                        [0, 9]]))
                        nc.vector.tensor_mul(
                            Pt[:, b0:b0 + nb, :],
                            NN[:, b0:b0 + nb, rho:rho + LC],
                            bass.AP(tensor=x_att.tensor,
                                    offset=(x_att.offset + ob * B * XAW
                                            + XAW * b0 + shoff_v),
                                    ap=[[NB * B * XAW, P], [XAW, nb],
                                        [65, RO], [1, HH]],
                                    dep_tracking_offset=(x_att.offset
                                                         + ob * B * XAW)))
                        for q in range(b0, b0 + nb):
                            for oo in range(NB):
                                nc.tensor.matmul(
                                    psF[oo][:, q * LC:(q + 1) * LC],
                                    wf16[(ob, oo)][:, :],
                                    Pt[:, q, :],
                                    start=(rho == 0 and ob == 0),
                                    stop=(rho == 8 and ob == NB - 1),
                                    tile_position=(0, 0),
                                    skip_group_check=True)

        # ================= phase 3: final conv + BNf ==================
        with ExitStack() as s3:
            fbuf = s3.enter_context(tc.tile_pool(name="fbuf", bufs=1))
            yf = [fbuf.tile([P, B * LC], F16, tag=f"yf{ob}",
                            name=f"yf{ob}") for ob in range(NB)]
            stf = persist.tile([P, 2, 4, 6], F32, tag="stf")
            msvf = persist.tile([P, 2, 2], F32, tag="msvf")
            # quarter-pipelined evict + stats (stats read PSUM directly,
            # in parallel with the Act evict)
            for q in range(B):
                sl = slice(q * LC, (q + 1) * LC)
                for ob in range(NB):
                    nc.scalar.activation(out=yf[ob][:, sl], in_=psF[ob][:, sl],
                                         func=AF.Copy)
                    nc.vector.bn_stats(out=stf[:, ob, q, :],
                                       in_=psF[ob][:, sl])
            for ob in range(NB):
                nc.vector.bn_aggr(out=msvf[:, ob, :], in_=stf[:, ob, :, :])
            nc.sync.dma_start(out=statd2[:, :],
                              in_=msvf[:, :, :].rearrange("p s v -> p (s v)"))
            nc.gpsimd.collective_compute(
                "AllGather", mybir.AluOpType.bypass,
                replica_groups=[list(range(NCORES))],
                ins=[statd2[:, :].opt()], outs=[statg2[:, :].opt()])
            sg2 = persist.tile([P, 4, 8], F32, tag="sg2")
            nc.sync.dma_start(
                out=sg2[:, :, :],
                in_=bass.AP(tensor=statg2.tensor, offset=statg2.offset,
                            ap=[[4, P], [1, 4], [4 * P, 8]]))
            gmean2 = persist.tile([P, 2], F32, tag="gmean2")
            gvar2 = persist.tile([P, 2], F32, tag="gvar2")
            combine_stats(sg2, 2, gmean2, gvar2)
            gq2 = persist.tile([P, 2], F32, tag="gq2")
            bq2 = persist.tile([P, 2], F32, tag="bq2")
            for ob in range(NB):
                nc.vector.tensor_copy(out=gq2[:, ob:ob + 1],
                                      in_=bnc[ob][:, 4:5])
                nc.vector.tensor_copy(out=bq2[:, ob:ob + 1],
                                      in_=bnc[ob][:, 5:6])
            acolf = persist.tile([P, 2], F32, tag="acolf")
            bcolf = persist.tile([P, 2], F32, tag="bcolf")
            bn_coeffs(gmean2, gvar2, acolf, bcolf, gq2[:, :], bq2[:, :])
            fout = [fbuf.tile([P, B * LC], F32, tag=f"fout{ob}",
                              name=f"fout{ob}") for ob in range(NB)]
            for b in range(B):
                sl = slice(b * LC, (b + 1) * LC)
                for ob in range(NB):
                    nc.scalar.activation(out=fout[ob][:, sl],
                                         in_=yf[ob][:, sl],
                                         func=AF.Relu,
                                         bias=bcolf[:, ob:ob + 1],
                                         scale=acolf[:, ob:ob + 1])
                    nc.sync.dma_start(out=out_t[ob * P:(ob + 1) * P, sl],
                                      in_=fout[ob][:, sl])

    nc.finalize()
    return nc


_NC_CACHE = None


def _get_nc():
    global _NC_CACHE
    if _NC_CACHE is None:
        _NC_CACHE = build_program()
    return _NC_CACHE


def make_in_maps(inputs):
    x = np.asarray(inputs["x"], np.float32).reshape(B, C, HH, HH)
    w1t = np.asarray(inputs["w1"], np.float32).T
    w2t = np.asarray(inputs["w2"], np.float32).T
    wft = np.asarray(inputs["wf"], np.float32).T
    wts = np.ascontiguousarray(
        np.concatenate([w1t, w2t, wft], axis=1))  # [256, 768]
    bnq = np.ascontiguousarray(np.stack([
        np.asarray(inputs["g1"], np.float32),
        np.asarray(inputs["be1"], np.float32),
        np.asarray(inputs["g2"], np.float32),
        np.asarray(inputs["be2"], np.float32),
        np.asarray(inputs["gf"], np.float32),
        np.asarray(inputs["bef"], np.float32),
    ], axis=1))  # [256, 6]

    shifts = [(_shift(t)) for t in range(9)]
    maps = []
    for k in range(NCORES):
        r0 = RO * k
        # xs slab: rows r0-2 .. r0+9, zero outside image
        xs = np.zeros((NB * P, CC), np.float32)
        for rr in range(RH):
            gr = r0 - 2 + rr
            if 0 <= gr < HH:
                # xs[ib*128+ci, b*768 + rr*64 + cc]
                blk = x[:, :, gr, :]  # [B, C, 64]
                for ib in range(NB):
                    xs[ib * P:(ib + 1) * P,
                       np.arange(B)[:, None] * 768 + rr * 64
                       + np.arange(HH)[None, :]] = \
                        blk[:, ib * P:(ib + 1) * P, :].transpose(1, 0, 2)
        # patch strips xp [2P, B*9*16] + validity mask hpmv [P, 144]
        xp = np.zeros((NB * P, NPC), np.float32)
        hpmv = np.zeros((P, 144), np.float16)
        for t in range(9):
            for jj in range(16):
                if jj < 8:
                    l = LC * k - 8 + jj
                    ts, lp = (t, l) if l >= 0 else (t - 1, l + L)
                else:
                    l = LC * k + LC + (jj - 8)
                    ts, lp = (t, l) if l < L else (t + 1, l - L)
                if ts < 0 or ts > 8:
                    continue
                di, dj = shifts[ts]
                rr_, cc_ = lp // HH + di, lp % HH + dj
                if 0 <= rr_ < HH and 0 <= cc_ < HH:
                    hpmv[:, t * 16 + jj] = 1.0
                    col = np.arange(B) * 144 + t * 16 + jj
                    val = x[:, :, rr_, cc_]  # [B, C]
                    for ib in range(NB):
                        xp[ib * P:(ib + 1) * P, col] = \
                            val[:, ib * P:(ib + 1) * P].T
        hmask = np.ones((P, 2), np.float32)
        if k == 0:
            hmask[:, 0] = 0.0
        if k == NCORES - 1:
            hmask[:, 1] = 0.0
        offt = np.zeros((1, 18), np.int32)
        for rho in range(9):
            t = (rho + k) % 9
            di, dj = shifts[t]
            offt[0, rho] = 66 + 65 * di + dj
            offt[0, 9 + rho] = 16 * t
        maps.append({
            "xs": xs, "xp": xp, "wts": wts,
            "bnq": bnq, "hmask": hmask, "offt": offt, "hpmv": hpmv,
        })
    return maps


def run(inputs, trace=False):
    nc = _get_nc()
    in_maps = make_in_maps(inputs)
    res = run_bass_kernel_spmd(nc, in_maps, core_ids=list(range(NCORES)),
                               trace=trace)
    full = np.empty((B, C, HH, HH), np.float32)
    for k in range(NCORES):
        o = res.results[k]["out"].reshape(NB, P, B, RO, HH)
        for ob in range(NB):
            full[:, ob * P:(ob + 1) * P, RO * k:RO * (k + 1), :] = \
                o[ob].transpose(1, 0, 2, 3)
    return full, res


def kernel(**inputs) -> np.ndarray:
    out, _ = run(inputs, trace=False)
    return out


# revision 5
# speedup vs baseline: 1.0162x; 1.0162x over previous
"""Trainium2 Bass kernel for nn_Dependence_Learning (sparse_attention) — v2.

L-sharded design: each of 8 cores owns 8 image rows (512 pixels) of every
(batch, channel) pair.  Channels live on partitions (2 banks of 128).

  conv1/conv2: lhsT = w.T [in_ch part, out_ch], rhs = x [in_ch part, cols]
  BN stats: per-channel = per-partition bn_stats/bn_aggr over the owned
  region, then ONE AllGather of (mean, var) x 4 slots and a local combine.
  tri maps stored fp16 in a 65-col padded row layout (shared pad column
  gives correct zero for +-1 column shifts).

  Attention: the reference's scrambled reshape makes softmax run over
  groups of 9 consecutive positions of the flat space p = 4096 t + l.
  Per core the group phase r = (t - k) mod 9.  The loop runs over the
  PHASE rho (static geometry on every core); the shift used is
  t = (rho + k) mod 9, supplied per-core as a register offset table
  ("offt") that biases the tri / x source APs.  Cross-shift seam values
  (first/last <=8 positions, cores 0/7) come from a host-prepared patch
  strip ("xp") run through the same conv+BN path ("hp"), patched into the
  F tiles with one strided copy.

  Engine split per (rho, ob): Act: F overcopies + seam patches + exp;
  Pool(gpsimd): the two center-broadcast muls (t2's last batch on DVE);
  DVE: add, group-sum, recip, normalize, x-mul, BN applies; PE: the
  final 1x1 conv FUSED into the loop (yf = wf . sum_t Pt is linear, so
  each Pt feeds wf-matmuls accumulating straight into the final PSUM).

  Then yf evict + bn_stats, second AllGather for BNf stats, relu, out.
"""

import numpy as np
from contextlib import ExitStack

import concourse.bass as bass
import concourse.bacc as bacc
import concourse.tile as tile
import concourse.mybir as mybir
from concourse.bass_utils import run_bass_kernel_spmd

F32 = mybir.dt.float32
F32R = mybir.dt.float32r
F16 = mybir.dt.float16
I32 = mybir.dt.int32
AF = mybir.ActivationFunctionType
AX = mybir.AxisListType

NCORES = 8
B = 4
C = 256
HH = 64
L = HH * HH            # 4096
P = 128
NB = 2                 # channel banks
RO = 8                 # owned rows per core
LC = RO * HH           # 512 owned pixels
RH = 12                # slab rows (halo 2 each side)
RX = 10                # x_att rows (halo 1 each side)
CC = B * RH * HH       # 3072 conv cols per bank
TRW = 1 + RH * 65      # 781   tri row-padded width per b
XAW = 1 + RX * 65      # 651
EPS = 1e-5
EXPB = -30.0
NPC = B * 9 * 16       # 576 patch-strip cols


def _shift(t):
    return t // 3 - 1, t % 3 - 1


def _geom(rho):
    s = (1 - rho) % 9
    ng = (rho + 512 + s) // 9
    return s, ng


def build_program():
    nc = bacc.Bacc("TRN2", target_bir_lowering=False, num_devices=NCORES)

    xs_t = nc.dram_tensor("xs", [NB * P, CC], F32R, kind="ExternalInput")
    xp_t = nc.dram_tensor("xp", [NB * P, NPC], F32R, kind="ExternalInput")
    wts_t = nc.dram_tensor("wts", [NB * P, 3 * C], F32R, kind="ExternalInput")
    bnq_t = nc.dram_tensor("bnq", [NB * P, 6], F32, kind="ExternalInput")
    hm_t = nc.dram_tensor("hmask", [P, 2], F32, kind="ExternalInput")
    hpm_t = nc.dram_tensor("hpmv", [P, 144], F16, kind="ExternalInput")
    offt_t = nc.dram_tensor("offt", [1, 18], I32, kind="ExternalInput")
    out_t = nc.dram_tensor("out", [NB * P, B * LC], F32, kind="ExternalOutput")

    with tile.TileContext(nc) as tc, ExitStack() as top:
        consts = top.enter_context(tc.tile_pool(name="consts", bufs=1))
        persist = top.enter_context(tc.tile_pool(name="persist", bufs=1))
        tiny = top.enter_context(tc.tile_pool(name="tiny", bufs=4))
        dram = top.enter_context(tc.tile_pool(name="dram", bufs=1, space="DRAM"))

        # ---- weights / constants ----
        # all conv weights in two [128, 768] tiles (one DMA each):
        # cols = (w1|w2|wf) * 256 + ob * 128 + co
        wbig = []
        for ib in range(NB):
            wb = consts.tile([P, 3 * C], F32R, tag=f"wbig{ib}",
                             name=f"wbig{ib}")
            nc.scalar.dma_start(out=wb[:, :],
                                in_=wts_t[ib * P:(ib + 1) * P, :])
            wbig.append(wb)
        _wi = {"w1": 0, "w2": 1, "wf": 2}

        def wt(key):
            name, ib, ob = key
            j = _wi[name] * C + ob * P
            return wbig[ib][:, j:j + P]
        # fp16 wf blocks for the fused final-conv accumulation
        wf16 = {}
        for ib in range(NB):
            for oo in range(NB):
                w16 = consts.tile([P, P], F16, tag=f"wf16_{ib}{oo}",
                                  name=f"wf16_{ib}{oo}")
                nc.vector.tensor_copy(out=w16[:, :],
                                      in_=wt(("wf", ib, oo)).bitcast(F32))
                wf16[(ib, oo)] = w16
        bnc = []
        for ob in range(NB):
            c_ = consts.tile([P, 6], F32, tag=f"bnc{ob}", name=f"bnc{ob}")
            nc.sync.dma_start(out=c_[:, :], in_=bnq_t[ob * P:(ob + 1) * P, :])
            bnc.append(c_)
        hmask = consts.tile([P, 2], F32, tag="hmask")
        nc.sync.dma_start(out=hmask[:, :], in_=hm_t[:, :])
        hpm = consts.tile([P, 144], F16, tag="hpm")
        nc.gpsimd.dma_start(out=hpm[:, :], in_=hpm_t[:, :])
        offs = consts.tile([1, 18], I32, tag="offs")
        nc.sync.dma_start(out=offs[:, :], in_=offt_t[:, :])
        eps_c = consts.tile([P, 1], F32, tag="eps_c")
        nc.vector.memset(eps_c[:, :], EPS)
        expb_c = consts.tile([P, 1], F32, tag="expb_c")
        nc.vector.memset(expb_c[:, :], EXPB)

        # ---- persistent attention-phase buffers ----
        tri = [persist.tile([P, NB, B, TRW], F16, tag=f"tri{m}",
                            name=f"tri{m}") for m in range(2)]
        x_att = persist.tile([P, NB, B, XAW], F16, tag="x_att")
        hp = [persist.tile([P, NB, B, 9, 16], F16, tag=f"hp{m}",
                           name=f"hp{m}") for m in range(2)]
        acol = persist.tile([P, 4], F32, tag="acol")
        bcol = persist.tile([P, 4], F32, tag="bcol")

        statd = dram.tile([P, 8], F32, tag="statd")
        statg = dram.tile([NCORES * P, 8], F32, tag="statg")
        statd2 = dram.tile([P, 4], F32, tag="statd2")
        statg2 = dram.tile([NCORES * P, 4], F32, tag="statg2")

        def combine_stats(sg, nslots, gmean, gvar):
            """sg [P, nslots(mean,var interleaved j), 8 cores] -> global."""
            msq = tiny.tile([P, nslots, 8], F32, tag="msq")
            mv = sg[:, :, :]  # [P, 2*nslots, 8]
            mean_v = bass.AP(tensor=sg.tensor, offset=sg.offset,
                             ap=[[2 * nslots * 8, P], [16, nslots], [1, 8]])
            var_v = bass.AP(tensor=sg.tensor, offset=sg.offset + 8,
                            ap=[[2 * nslots * 8, P], [16, nslots], [1, 8]])
            nc.vector.tensor_mul(msq[:, :, :], mean_v, mean_v)
            nc.vector.tensor_add(msq[:, :, :], msq[:, :, :], var_v)
            nc.vector.reduce_sum(out=gmean[:, :], in_=mean_v, axis=AX.X)
            nc.vector.reduce_sum(out=gvar[:, :], in_=msq[:, :, :], axis=AX.X)
            nc.vector.tensor_scalar_mul(gmean[:, :], gmean[:, :], 1.0 / 8)
            nc.vector.tensor_scalar_mul(gvar[:, :], gvar[:, :], 1.0 / 8)
            gm2 = tiny.tile([P, nslots], F32, tag="gm2")
            nc.vector.tensor_mul(gm2[:, :], gmean[:, :], gmean[:, :])
            nc.vector.tensor_sub(gvar[:, :], gvar[:, :], gm2[:, :])

        def bn_coeffs(gmean, gvar, acol_, bcol_, gcols, becols):
            """acol = g/sqrt(var+eps); bcol = be - mean*acol (all [P, n])."""
            n = gvar.shape[1]
            sd = tiny.tile([P, n], F32, tag="sd")
            nc.scalar.activation(out=sd[:, :], in_=gvar[:, :], func=AF.Sqrt,
                                 bias=eps_c[:, :])
            nc.vector.reciprocal(sd[:, :], sd[:, :])
            nc.vector.tensor_mul(acol_[:, :], gcols, sd[:, :])
            tmp = tiny.tile([P, n], F32, tag="tmpc")
            nc.vector.tensor_mul(tmp[:, :], gmean[:, :], acol_[:, :])
            nc.vector.tensor_sub(bcol_[:, :], becols, tmp[:, :])

        # ================= phase 1: convs + BN stats ==================
        with ExitStack() as s1:
            rhsp = s1.enter_context(tc.tile_pool(name="rhs", bufs=4))
            psump = s1.enter_context(
                tc.tile_pool(name="psum", bufs=4, space="PSUM"))
            ybuf = s1.enter_context(tc.tile_pool(name="ybuf", bufs=1))
            y = {}
            for m in range(2):
                for ob in range(NB):
                    y[(m, ob)] = ybuf.tile([P, CC], F16, tag=f"y{m}{ob}",
                                           name=f"y{m}{ob}")
            hpraw = [ybuf.tile([P, NB, B, 9, 16], F16, tag=f"hpr{m}",
                               name=f"hpr{m}") for m in range(2)]

            xc = [rhsp.tile([P, CC], F32R, tag=f"xc{ib}", name=f"xc{ib}")
                  for ib in range(NB)]
            for ib in range(NB):
                for h in range(3):
                    nc.sync.dma_start(
                        out=xc[ib][:, h * 1024:(h + 1) * 1024],
                        in_=xs_t[ib * P:(ib + 1) * P,
                                 h * 1024:(h + 1) * 1024])

            # owned-region chunks first (contiguous 512 cols per b) so BN
            # stats finish before the halo columns; the halo conv then
            # overlaps the stats AllGather.
            CW = 512
            for b in range(B):
                sl = slice(b * 768 + 128, b * 768 + 640)
                for m, wname in ((0, "w1"), (1, "w2")):
                    for ob in range(NB):
                        ps = psump.tile([P, CW], F32, tag="ps", name="ps")
                        for ib in range(NB):
                            nc.tensor.matmul(
                                ps[:, :], wt((wname, ib, ob)),
                                xc[ib][:, sl], start=(ib == 0), stop=(ib == 1),
                                tile_position=(0, 0))
                        nc.scalar.activation(
                            out=y[(m, ob)][:, sl],
                            in_=ps[:, :], func=AF.Copy)
            # halo columns: 128 cols at each end of every b-block, strided
            for h in range(2):
                for m, wname in ((0, "w1"), (1, "w2")):
                    for ob in range(NB):
                        ps = psump.tile([P, CW], F32, tag="ps", name="ps")
                        for ib in range(NB):
                            nc.tensor.matmul(
                                ps[:, :], wt((wname, ib, ob)),
                                bass.AP(tensor=xc[ib].tensor,
                                        offset=xc[ib].offset + h * 640,
                                        ap=[[CC, P], [768, B], [1, 128]]),
                                start=(ib == 0), stop=(ib == 1),
                                tile_position=(0, 0))
                        nc.scalar.activation(
                            out=bass.AP(
                                tensor=y[(m, ob)].tensor,
                                offset=y[(m, ob)].offset + h * 640,
                                ap=[[CC, P], [768, B], [1, 128]]),
                            in_=ps[:, :].rearrange("p (b c) -> p b c", b=B),
                            func=AF.Copy)

            # x_att: cast slab rows 1..10 into padded layout (early: only
            # needs xc), plus all pad-position zeroing for x_att and tri
            for ib in range(NB):
                nc.vector.tensor_copy(
                    out=bass.AP(
                        tensor=x_att.tensor,
                        offset=x_att.offset + ib * B * XAW + 1,
                        ap=[[NB * B * XAW, P], [XAW, B], [65, RX], [1, HH]]),
                    in_=bass.AP(tensor=xc[ib].tensor,
                                offset=xc[ib].offset + HH,
                                ap=[[CC, P], [768, B], [64, RX],
                                    [1, HH]]).bitcast(F32))
            nc.vector.memset(
                bass.AP(tensor=x_att.tensor, offset=x_att.offset,
                        ap=[[NB * B * XAW, P], [B * XAW, NB], [XAW, B],
                            [65, RX + 1]]), 0.0)
            for m in range(2):
                nc.vector.memset(
                    bass.AP(tensor=tri[m].tensor, offset=tri[m].offset,
                            ap=[[NB * B * TRW, P], [B * TRW, NB], [TRW, B],
                                [65, RH + 1]]), 0.0)

            # patch strips: same convs on xp
            rp = []
            for ib in range(NB):
                r = rhsp.tile([P, NPC], F32R, tag="rhsp", name=f"rhsp{ib}")
                nc.gpsimd.dma_start(out=r[:, :],
                                    in_=xp_t[ib * P:(ib + 1) * P, :])
                rp.append(r)
            for m, wname in ((0, "w1"), (1, "w2")):
                for ob in range(NB):
                    for half in range(2):
                        sl = slice(half * 288, (half + 1) * 288)
                        ps = psump.tile([P, 288], F32, tag="psp", name="psp")
                        for ib in range(NB):
                            nc.tensor.matmul(
                                ps[:, :], wt((wname, ib, ob)),
                                rp[ib][:, sl], start=(ib == 0), stop=(ib == 1),
                                tile_position=(0, 0))
                        nc.scalar.activation(
                            out=hpraw[m][:, ob, 2 * half:2 * half + 2]
                            .rearrange("p b t j -> p (b t j)"),
                            in_=ps[:, :], func=AF.Copy)

            # ---- local BN stats ----
            stt = persist.tile([P, 4, 4, 6], F32, tag="stt")
            msv = persist.tile([P, 4, 2], F32, tag="msv")
            for m in range(2):
                for ob in range(NB):
                    slot = 2 * m + ob
                    for b in range(B):
                        nc.vector.bn_stats(
                            out=stt[:, slot, b, :],
                            in_=y[(m, ob)][:, b * 768 + 128:b * 768 + 640])
                    nc.vector.bn_aggr(out=msv[:, slot, :],
                                      in_=stt[:, slot, :, :])
            nc.sync.dma_start(out=statd[:, :],
                              in_=msv[:, :, :].rearrange("p s v -> p (s v)"))
            nc.gpsimd.collective_compute(
                "AllGather", mybir.AluOpType.bypass,
                replica_groups=[list(range(NCORES))],
                ins=[statd[:, :].opt()], outs=[statg[:, :].opt()])
            sg = persist.tile([P, 8, 8], F32, tag="sg")
            nc.sync.dma_start(
                out=sg[:, :, :],
                in_=bass.AP(tensor=statg.tensor, offset=statg.offset,
                            ap=[[8, P], [1, 8], [8 * P, 8]]))
            gmean = persist.tile([P, 4], F32, tag="gmean")
            gvar = persist.tile([P, 4], F32, tag="gvar")
            combine_stats(sg, 4, gmean, gvar)
            # g/be columns per slot: slot=2m+ob -> bnc[ob][:, 2m], [:, 2m+1]
            gq = persist.tile([P, 4], F32, tag="gq")
            bq = persist.tile([P, 4], F32, tag="bq")
            for m in range(2):
                for ob in range(NB):
                    slot = 2 * m + ob
                    nc.vector.tensor_copy(out=gq[:, slot:slot + 1],
                                          in_=bnc[ob][:, 2 * m:2 * m + 1])
                    nc.vector.tensor_copy(out=bq[:, slot:slot + 1],
                                          in_=bnc[ob][:, 2 * m + 1:2 * m + 2])
            bn_coeffs(gmean, gvar, acol, bcol, gq[:, :], bq[:, :])

            # ---- BN apply + relu -> tri (padded), hp ----
            # Halo rows at the global image edge must be 0 (reference pads
            # tri with zeros).  relu(hm*(a*y+b)) == hm*relu(a*y+b) for
            # hm in {0,1}, so masked coefficients fold the mask into the
            # apply: 3 row bands (top halo, interior, bottom halo).
            acolH = persist.tile([P, 4], F32, tag="acolH")
            bcolH = persist.tile([P, 4], F32, tag="bcolH")
            acolB = persist.tile([P, 4], F32, tag="acolB")
            bcolB = persist.tile([P, 4], F32, tag="bcolB")
            nc.vector.tensor_scalar_mul(acolH[:, :], acol[:, :], hmask[:, 0:1])
            nc.vector.tensor_scalar_mul(bcolH[:, :], bcol[:, :], hmask[:, 0:1])
            nc.vector.tensor_scalar_mul(acolB[:, :], acol[:, :], hmask[:, 1:2])
            nc.vector.tensor_scalar_mul(bcolB[:, :], bcol[:, :], hmask[:, 1:2])
            for ob in range(NB):
                for m in range(2):
                    slot = 2 * m + ob
                    # BN apply on DVE (tensor_scalar affine at fp16 4x,
                    # then relu) so Act is free for the first overcopies
                    for (r0_, nr_, ac_, bc_) in (
                            (2, RH - 4, acol, bcol),
                            (0, 2, acolH, bcolH),
                            (RH - 2, 2, acolB, bcolB)):
                        dst = bass.AP(
                            tensor=tri[m].tensor,
                            offset=(tri[m].offset + ob * B * TRW + 1
                                    + r0_ * 65),
                            ap=[[NB * B * TRW, P], [TRW, B], [65, nr_],
                                [1, HH]])
                        nc.vector.tensor_scalar(
                            out=dst,
                            in0=bass.AP(
                                tensor=y[(m, ob)].tensor,
                                offset=y[(m, ob)].offset + r0_ * HH,
                                ap=[[CC, P], [768, B], [64, nr_], [1, HH]]),
                            scalar1=ac_[:, slot:slot + 1],
                            scalar2=bc_[:, slot:slot + 1],
                            op0=mybir.AluOpType.mult,
                            op1=mybir.AluOpType.add)
                        nc.vector.tensor_scalar_max(dst, dst, 0.0)
                    nc.scalar.activation(
                        out=hp[m][:, ob], in_=hpraw[m][:, ob], func=AF.Relu,
                        bias=bcol[:, slot:slot + 1],
                        scale=acol[:, slot:slot + 1])
                    # zero the patch slots whose source pixel is outside
                    # the image (reference zero-padding)
                    nc.vector.tensor_mul(
                        hp[m][:, ob],
                        hp[m][:, ob],
                        bass.AP(tensor=hpm.tensor, offset=hpm.offset,
                                ap=[[144, P], [0, B], [16, 9], [1, 16]]))
        # ================= phase 2: attention ==================
        # psF accumulates the FINAL conv directly: yf = wf . sum_t Pt
        # (linear), so each iteration's Pt feeds wf-matmuls straight into
        # the final-conv PSUM — no separate pre accumulation/eviction.
        psumA = top.enter_context(
            tc.tile_pool(name="psumA", bufs=1, space="PSUM"))
        psF = [psumA.tile([P, B * LC], F32, tag=f"psF{oo}",
                          name=f"psF{oo}") for oo in range(NB)]
        with ExitStack() as s2:
            fpool = s2.enter_context(tc.tile_pool(name="fpool", bufs=3))
            tpool = s2.enter_context(tc.tile_pool(name="tpool", bufs=3))
            npool = s2.enter_context(tc.tile_pool(name="npool", bufs=3))

            for rho in range(9):
                s_, ng = _geom(rho)
                w9 = 9 * ng
                # per-core shift offsets (registers)
                shoff_a = nc.scalar.value_load(offs[0:1, rho:rho + 1])
                shoff_v = nc.vector.value_load(offs[0:1, rho:rho + 1])
                hoff_a = nc.scalar.value_load(offs[0:1, 9 + rho:10 + rho])
                for ob in range(NB):
                    Fc = []
                    for m in range(2):
                        F_ = fpool.tile([P, B, 640], F16, tag=f"F{m}",
                                        name=f"F{m}c")
                        # overcopy: 10 shifted rows per b (Act)
                        src = bass.AP(
                            tensor=tri[m].tensor,
                            offset=tri[m].offset + ob * B * TRW + shoff_a,
                            ap=[[NB * B * TRW, P], [TRW, B], [65, 10],
                                [1, 64]],
                            dep_tracking_offset=(tri[m].offset
                                                 + ob * B * TRW))
                        dst = F_[:, :, :].rearrange(
                            "p b (r c) -> p b r c", c=64)
                        nc.scalar.activation(out=dst, in_=src, func=AF.Copy)
                        # seam patch (Act): head [56:64) + tail [576:584)
                        nc.scalar.activation(
                            out=bass.AP(tensor=F_.tensor,
                                        offset=F_.offset + 56,
                                        ap=[[B * 640, P], [640, B], [520, 2],
                                            [1, 8]]),
                            in_=bass.AP(
                                tensor=hp[m].tensor,
                                offset=hp[m].offset + ob * B * 144 + hoff_a,
                                ap=[[NB * B * 144, P], [144, B], [8, 2],
                                    [1, 8]],
                                dep_tracking_offset=(hp[m].offset
                                                     + ob * B * 144)),
                            func=AF.Copy)
                        Fc.append(F_)
                    t1 = tpool.tile([P, B, 528], F16, tag="t1", name="t1")
                    t2 = tpool.tile([P, B, 528], F16, tag="t2", name="t2")
                    lgb = tpool.tile([P, B, 528], F16, tag="lgb", name="lgb")
                    Ne = npool.tile([P, B, 528], F32, tag="Ne", name="Ne")
                    Z = npool.tile([P, B, 64], F32, tag="Z", name="Z")
                    R = npool.tile([P, B, 64], F32, tag="R", name="R")
                    NN = npool.tile([P, B, 528], F16, tag="NN", name="NN")
                    Pt = npool.tile([P, B, LC], F16, tag="Pt", name="Pt")

                    def gv(tl, off, b0, nb):
                        return bass.AP(
                            tensor=tl.tensor,
                            offset=tl.offset + tl.shape[2] * b0 + off,
                            ap=[[tl.shape[1] * tl.shape[2], P],
                                [tl.shape[2], nb], [9, ng], [1, 9]])

                    def cbv(tl, off, b0, nb):
                        return bass.AP(
                            tensor=tl.tensor,
                            offset=tl.offset + tl.shape[2] * b0 + off,
                            ap=[[tl.shape[1] * tl.shape[2], P],
                                [tl.shape[2], nb], [9, ng], [0, 9]])

                    # last phase runs per-b to shorten the drain chain
                    bsl = [(b, 1) for b in range(B)] if rho == 8 else [(0, B)]
                    for (b0, nb) in bsl:
                        # center-broadcast muls: t1 on Pool; t2 split
                        # Pool (b<3) / DVE (b=3) to balance the pacer
                        nc.gpsimd.tensor_mul(
                            gv(t1, 0, b0, nb), gv(Fc[0], 64 - rho, b0, nb),
                            cbv(Fc[1], 64 - rho + 4, b0, nb))
                        for (c0, nb2, eng) in (((b0, min(nb, 3), nc.gpsimd),
                                                (3, 1, nc.vector))
                                               if (b0 == 0 and nb == B) else
                                               ((b0, nb,
                                                 nc.vector if b0 == 3
                                                 else nc.gpsimd),)):
                            eng.tensor_mul(
                                gv(t2, 0, c0, nb2),
                                gv(Fc[1], 64 - rho, c0, nb2),
                                cbv(Fc[0], 64 - rho + 4, c0, nb2))
                        nc.vector.tensor_add(
                            lgb[:, b0:b0 + nb, :w9],
                            t1[:, b0:b0 + nb, :w9], t2[:, b0:b0 + nb, :w9])
                        nc.scalar.activation(out=Ne[:, b0:b0 + nb, :w9],
                                             in_=lgb[:, b0:b0 + nb, :w9],
                                             func=AF.Exp, bias=expb_c[:, :])
                        nc.vector.reduce_sum(
                            out=Z[:, b0:b0 + nb, :ng],
                            in_=Ne[:, b0:b0 + nb, :w9].rearrange(
                                "p b (g s) -> p b g s", s=9),
                            axis=AX.X)
                        nc.vector.reciprocal(R[:, b0:b0 + nb, :ng],
                                             Z[:, b0:b0 + nb, :ng])
                        nc.vector.tensor_mul(
                            NN[:, b0:b0 + nb, :w9].rearrange(
                                "p b (g s) -> p b g s", s=9),
                            Ne[:, b0:b0 + nb, :w9].rearrange(
                                "p b (g s) -> p b g s", s=9),
                            bass.AP(tensor=R.tensor,
                                    offset=R.offset + 64 * b0,
                                    ap=[[B * 64, P], [64, nb], [1, ng],
                                        [0, 9]]))
                        nc.vector.tensor_mul(
                            Pt[:, b0:b0 + nb, :],
                            NN[:, b0:b0 + nb, rho:rho + LC],
                            bass.AP(tensor=x_att.tensor,
                                    offset=(x_att.offset + ob * B * XAW
                                            + XAW * b0 + shoff_v),
                                    ap=[[NB * B * XAW, P], [XAW, nb],
                                        [65, RO], [1, HH]],
                                    dep_tracking_offset=(x_att.offset
                                                         + ob * B * XAW)))
                        for q in range(b0, b0 + nb):
                            for oo in range(NB):
                                nc.tensor.matmul(
                                    psF[oo][:, q * LC:(q + 1) * LC],
                                    wf16[(ob, oo)][:, :],
                                    Pt[:, q, :],
                                    start=(rho == 0 and ob == 0),
                                    stop=(rho == 8 and ob == NB - 1),
                                    tile_position=(0, 0),
                                    skip_group_check=True)

        # ================= phase 3: final conv + BNf ==================
        with ExitStack() as s3:
            fbuf = s3.enter_context(tc.tile_pool(name="fbuf", bufs=1))
            yf = [fbuf.tile([P, B * LC], F16, tag=f"yf{ob}",
                            name=f"yf{ob}") for ob in range(NB)]
            stf = persist.tile([P, 2, 4, 6], F32, tag="stf")
            msvf = persist.tile([P, 2, 2], F32, tag="msvf")
            # quarter-pipelined evict + stats (stats read PSUM directly,
            # in parallel with the Act evict)
            for q in range(B):
                sl = slice(q * LC, (q + 1) * LC)
                for ob in range(NB):
                    nc.scalar.activation(out=yf[ob][:, sl], in_=psF[ob][:, sl],
                                         func=AF.Copy)
                    nc.vector.bn_stats(out=stf[:, ob, q, :],
                                       in_=psF[ob][:, sl])
            for ob in range(NB):
                nc.vector.bn_aggr(out=msvf[:, ob, :], in_=stf[:, ob, :, :])
            nc.sync.dma_start(out=statd2[:, :],
                              in_=msvf[:, :, :].rearrange("p s v -> p (s v)"))
            nc.gpsimd.collective_compute(
                "AllGather", mybir.AluOpType.bypass,
                replica_groups=[list(range(NCORES))],
                ins=[statd2[:, :].opt()], outs=[statg2[:, :].opt()])
            sg2 = persist.tile([P, 4, 8], F32, tag="sg2")
            nc.sync.dma_start(
                out=sg2[:, :, :],
                in_=bass.AP(tensor=statg2.tensor, offset=statg2.offset,
                            ap=[[4, P], [1, 4], [4 * P, 8]]))
            gmean2 = persist.tile([P, 2], F32, tag="gmean2")
            gvar2 = persist.tile([P, 2], F32, tag="gvar2")
            combine_stats(sg2, 2, gmean2, gvar2)
            gq2 = persist.tile([P, 2], F32, tag="gq2")
            bq2 = persist.tile([P, 2], F32, tag="bq2")
            for ob in range(NB):
                nc.vector.tensor_copy(out=gq2[:, ob:ob + 1],
                                      in_=bnc[ob][:, 4:5])
                nc.vector.tensor_copy(out=bq2[:, ob:ob + 1],
                                      in_=bnc[ob][:, 5:6])
            acolf = persist.tile([P, 2], F32, tag="acolf")
            bcolf = persist.tile([P, 2], F32, tag="bcolf")
            bn_coeffs(gmean2, gvar2, acolf, bcolf, gq2[:, :], bq2[:, :])
            fout = [fbuf.tile([P, B * LC], F32, tag=f"fout{ob}",
                              name=f"fout{ob}") for ob in range(NB)]
            for b in range(B):
                sl = slice(b * LC, (b + 1) * LC)
                for ob in range(NB):
                    nc.scalar.activation(out=fout[ob][:, sl],
                                         in_=yf[ob][:, sl],
                                         func=AF.Relu,
                                         bias=bcolf[:, ob:ob + 1],
                                         scale=acolf[:, ob:ob + 1])
                    nc.sync.dma_start(out=out_t[ob * P:(ob + 1) * P, sl],
                                      in_=fout[ob][:, sl])

    nc.finalize()
    return nc


_NC_CACHE = None


def _get_nc():
    global _NC_CACHE
    if _NC_CACHE is None:
        _NC_CACHE = build_program()
    return _NC_CACHE


def make_in_maps(inputs):
    x = np.asarray(inputs["x"], np.float32).reshape(B, C, HH, HH)
    w1t = np.asarray(inputs["w1"], np.float32).T
    w2t = np.asarray(inputs["w2"], np.float32).T
    wft = np.asarray(inputs["wf"], np.float32).T
    wts = np.ascontiguousarray(
        np.concatenate([w1t, w2t, wft], axis=1))  # [256, 768]
    bnq = np.ascontiguousarray(np.stack([
        np.asarray(inputs["g1"], np.float32),
        np.asarray(inputs["be1"], np.float32),
        np.asarray(inputs["g2"], np.float32),
        np.asarray(inputs["be2"], np.float32),
        np.asarray(inputs["gf"], np.float32),
        np.asarray(inputs["bef"], np.float32),
    ], axis=1))  # [256, 6]

    shifts = [(_shift(t)) for t in range(9)]
    maps = []
    for k in range(NCORES):
        r0 = RO * k
        # xs slab: rows r0-2 .. r0+9, zero outside image
        xs = np.zeros((NB * P, CC), np.float32)
        for rr in range(RH):
            gr = r0 - 2 + rr
            if 0 <= gr < HH:
                # xs[ib*128+ci, b*768 + rr*64 + cc]
                blk = x[:, :, gr, :]  # [B, C, 64]
                for ib in range(NB):
                    xs[ib * P:(ib + 1) * P,
                       np.arange(B)[:, None] * 768 + rr * 64
                       + np.arange(HH)[None, :]] = \
                        blk[:, ib * P:(ib + 1) * P, :].transpose(1, 0, 2)
        # patch strips xp [2P, B*9*16] + validity mask hpmv [P, 144]
        xp = np.zeros((NB * P, NPC), np.float32)
        hpmv = np.zeros((P, 144), np.float16)
        for t in range(9):
            for jj in range(16):
                if jj < 8:
                    l = LC * k - 8 + jj
                    ts, lp = (t, l) if l >= 0 else (t - 1, l + L)
                else:
                    l = LC * k + LC + (jj - 8)
                    ts, lp = (t, l) if l < L else (t + 1, l - L)
                if ts < 0 or ts > 8:
                    continue
                di, dj = shifts[ts]
                rr_, cc_ = lp // HH + di, lp % HH + dj
                if 0 <= rr_ < HH and 0 <= cc_ < HH:
                    hpmv[:, t * 16 + jj] = 1.0
                    col = np.arange(B) * 144 + t * 16 + jj
                    val = x[:, :, rr_, cc_]  # [B, C]
                    for ib in range(NB):
                        xp[ib * P:(ib + 1) * P, col] = \
                            val[:, ib * P:(ib + 1) * P].T
        hmask = np.ones((P, 2), np.float32)
        if k == 0:
            hmask[:, 0] = 0.0
        if k == NCORES - 1:
            hmask[:, 1] = 0.0
        offt = np.zeros((1, 18), np.int32)
        for rho in range(9):
            t = (rho + k) % 9
            di, dj = shifts[t]
            offt[0, rho] = 66 + 65 * di + dj
            offt[0, 9 + rho] = 16 * t
        maps.append({
            "xs": xs, "xp": xp, "wts": wts,
            "bnq": bnq, "hmask": hmask, "offt": offt, "hpmv": hpmv,
        })
    return maps


def run(inputs, trace=False):
    nc = _get_nc()
    in_maps = make_in_maps(inputs)
    res = run_bass_kernel_spmd(nc, in_maps, core_ids=list(range(NCORES)),
                               trace=trace)
    full = np.empty((B, C, HH, HH), np.float32)
    for k in range(NCORES):
        o = res.results[k]["out"].reshape(NB, P, B, RO, HH)
        for ob in range(NB):
            full[:, ob * P:(ob + 1) * P, RO * k:RO * (k + 1), :] = \
                o[ob].transpose(1, 0, 2, 3)
    return full, res


def kernel(**inputs) -> np.ndarray:
    out, _ = run(inputs, trace=False)
    return out
